# revision 1
# baseline (speedup 1.0000x reference)
"""Trainium2 Bass kernel for nn_CapsuleLayer (dynamic-routing capsule layer).

Math (reassociated so priors [K,A,R,D] are never materialized):
    stage(x_b, W, K, R):  # per (k, a) routing over R routes
      L = 0
      for it in 0..2:
        P = softmax_R(L)                          # uniform at it=0
        s[k,a,:] = sum_r P[k,a,r] * x_b[a,r,:]    # (B)  PE
        O_pre    = s @ W[k]                       # (A)  PE
        O        = O_pre * F(sq),  sq = sum_e O_pre^2   (squash)
        if it<2:
          V[k,a,:] = W[k] @ O[k,a,:]              # (V)  PE
          L[k,a,r] += sum_d x_b[a,r,d] V[k,a,d]   # (L)  PE
      return O

Sharding: 8 cores = 4 h-groups x 2 a-groups.
  Stage 1: h_loc=16 capsules, a_loc=32 batch, R=128 routes.
  AllGather of out1 among the 4 cores sharing an a-group.
  Stage 2: c_loc=8, a_loc=32, R=64 (=H) routes.
All matmuls fp32 (reduced precision breaks this net: routing softmax is
saturated, |logits| ~ 450, winner-take-all).

Layouts per core (f32):
  xn  [AL,128,256]        x natural; B-lhsT slices [r, d-half]
  xt  [AL,2,128,128]      x transposed per a, d-half: [d, r]; L-rhs
  w1n [HL,2,128,256]      W1 d-half-split: A-rhs [d,e]
  w1t [HL,2,2,128,128]    W1^T subtiles [eh][dh]: [e,d]; V-lhsT
  w2n [CL,2,128,256], w2t [CL,2,2,128,128]
Routing state q-layout: quad q=a//4, j=a%4, partition = 32*j + k (k<K).
"""

import os
import numpy as np

import concourse.bass as bass
from concourse import bacc
import concourse.mybir as mybir
import concourse.tile as tile
from concourse.masks import make_identity
from concourse.vector_clock import ScopedClock
from concourse.bass_utils import run_bass_kernel_spmd

F32 = mybir.dt.float32
AF = mybir.ActivationFunctionType
AX = mybir.AxisListType

A, B, D = 64, 128, 256
H, C = 64, 32
NIT = 3
PH, PA = 4, 2
HL, AL, CL = H // PH, A // PA, C // PH   # 16, 32, 8
N_CORES = PH * PA
R1, R2 = B, H                            # routes per stage


class _TC(tile.TileContext):
    """TileContext whose kernel-tail drain splits its sem waits across
    several SP instructions (this walrus build rejects >1 wait on a CTRL
    instruction: 'Too many sync wait commands')."""

    def _drain_and_barrier(self, tick_clock, wait_clock):
        drain_inst = self.nc.sync.drain()
        wait_clock.add_sem_waits(
            drain_inst.ins, ScopedClock({None: tick_clock.global_clock})
        )
        si = drain_inst.ins.sync_info
        waits = list(si.on_wait) if si and si.on_wait else []
        if len(waits) > 1:
            si.on_wait = waits[:1]
            for w in waits[1:]:
                nop = self.nc.sync.nop(hint="tail_wait_split")
                nsi = nop.ins.sync_info
                if nsi is None:
                    nop.ins.sync_info = mybir.SyncInfo(on_wait=[w], on_update=[])
                else:
                    nsi.on_wait = [w]
        self.nc.all_engine_barrier()
        popped = self.nc._tile_sem_poison_stack.pop()
        assert popped is self._sem_poison
        self.nc.clear_and_free_semaphores(list(self.sems.allocated().values()))
        self.nc.all_engine_barrier()


def _route_stage(nc, tc, ctx, pools, K, R, ident, uni,
                 b_lhsT, l_rhs, wn_sb, wt_sb, state):
    """Emit one routing stage. K capsules, R routes, AL batch.

    b_lhsT(a, dh) -> AP [R, 128]  (x natural slice; B stationary)
    l_rhs(a, dh)  -> AP [128, R]  (x^T slice; L moving)
    wn_sb[:, k, dh, :]    [128, 256]
    wt_sb[:, k, eh, dh, :] [128, 128]
    state: dict of persistent SBUF tiles for this stage.
    Returns O_sb tile ([128, ceil(K/4), 256]; rows 32*(k%4)+a', col-group k//4).
    """
    psum_big, psum_o, psum_sm, sb_small = pools
    NQ = AL // 4                   # quads of a
    NG = (K + 3) // 4              # k subgroups of 4
    L_sb = state["L"]              # [128, NQ, R]
    P_sb = state["P"]              # [128, NQ, R]
    PT_sb = state["PT"]            # [R, NQ, 128]
    S_sb = state["S"]              # [128, 2, AL*K]
    V_sb = state["V"]              # [128, 2, K, AL]
    O_sb = state["O"]              # [128, NG, 256]
    SQ = state["SQ"]               # [AL, K]
    F = state["F"]                 # [AL, K]

    nc.vector.memset(L_sb, 0.0)

    for it in range(NIT):
        last = it == NIT - 1
        # ---- probs ----
        if it > 0:
            for q in range(NQ):
                mx = sb_small.tile([128, 1], F32, tag="mx", name="mx")
                nmx = sb_small.tile([128, 1], F32, tag="nmx", name="nmx")
                sm = sb_small.tile([128, 1], F32, tag="sm", name="sm")
                rc = sb_small.tile([128, 1], F32, tag="rc", name="rc")
                nc.vector.reduce_max(out=mx, in_=L_sb[:, q, :], axis=AX.X)
                nc.vector.tensor_scalar_mul(nmx, mx, -1.0)
                nc.scalar.activation(out=P_sb[:, q, :], in_=L_sb[:, q, :],
                                     func=AF.Exp, bias=nmx, scale=1.0,
                                     accum_out=sm)
                nc.vector.reciprocal(rc, sm)
                nc.vector.tensor_scalar_mul(P_sb[:, q, :], P_sb[:, q, :], rc)
                pt_ps = psum_sm.tile([R, 128], F32, tag="t", name="t")
                nc.tensor.transpose(pt_ps, P_sb[:, q, :], ident)
                nc.vector.tensor_copy(PT_sb[:, q, :], pt_ps)

        # ---- B: S[dh][d, a*K+k] ----
        s_ps = psum_big.tile([128, 2, AL * K], F32, tag="big", name="big")
        for a in range(AL):
            q, j = a // 4, a % 4
            for dh in range(2):
                rhs = uni[:, :K] if it == 0 else PT_sb[:, q, 32 * j:32 * j + K]
                nc.tensor.matmul(s_ps[:, dh, a * K:(a + 1) * K],
                                 b_lhsT(a, dh), rhs,
                                 start=True, stop=True, skip_group_check=True)
        for dh in range(2):
            nc.vector.tensor_copy(S_sb[:, dh, :], s_ps[:, dh, :])
        S3 = S_sb.rearrange("p two (a k) -> p two a k", k=K)

        # ---- A + squash ----
        o_tiles = {}
        for g in range(NG):
            ks = [k for k in range(4 * g, min(4 * g + 4, K))]
            for k in ks:
                o_ps = psum_o.tile([AL, 256], F32, tag="o", name="o")
                o_tiles[k] = o_ps
                for dh in range(2):
                    nc.tensor.matmul(o_ps, S3[:, dh, :, k], wn_sb[:, k, dh, :],
                                     start=(dh == 0), stop=(dh == 1))
                scr = sb_small.tile([AL, 256], F32, tag="scr", name="scr")
                nc.scalar.activation(out=scr, in_=o_ps, func=AF.Square,
                                     accum_out=SQ[:, k:k + 1])
            g4 = slice(4 * g, 4 * g + len(ks))
            srt = sb_small.tile([AL, 4], F32, tag="srt", name="srt")
            sq1 = sb_small.tile([AL, 4], F32, tag="sq1", name="sq1")
            den = sb_small.tile([AL, 4], F32, tag="den", name="den")
            rcf = sb_small.tile([AL, 4], F32, tag="rcf", name="rcf")
            n = len(ks)
            nc.scalar.activation(out=srt[:, :n], in_=SQ[:, g4], func=AF.Sqrt)
            nc.vector.tensor_scalar_add(sq1[:, :n], SQ[:, g4], 1.0)
            nc.vector.tensor_mul(den[:, :n], sq1[:, :n], srt[:, :n])
            nc.vector.reciprocal(rcf[:, :n], den[:, :n])
            nc.vector.tensor_mul(F[:, g4], SQ[:, g4], rcf[:, :n])
            for k in ks:
                s = k % 4
                nc.scalar.activation(out=O_sb[32 * s:32 * s + AL, g, :],
                                     in_=o_tiles[k], func=AF.Copy,
                                     scale=F[:, k:k + 1])

        if last:
            break

        # ---- OT + V ----
        v_ps = psum_big.tile([128, 2, K * AL], F32, tag="big", name="big")
        for k in range(K):
            s, g = k % 4, k // 4
            ot = []
            for eh in range(2):
                t_ps = psum_sm.tile([128, AL], F32, tag="t", name="t")
                nc.tensor.transpose(
                    t_ps, O_sb[32 * s:32 * s + AL, g, 128 * eh:128 * eh + 128],
                    ident[32 * s:32 * s + AL, 32 * s:32 * s + AL],
                    tile_position=(32 * s, 0))
                ot_sb = sb_small.tile([128, AL], F32, tag="ot", name="ot")
                nc.vector.tensor_copy(ot_sb, t_ps)
                ot.append(ot_sb)
            for dh in range(2):
                for eh in range(2):
                    nc.tensor.matmul(v_ps[:, dh, k * AL:(k + 1) * AL],
                                     wt_sb[:, k, eh, dh, :], ot[eh],
                                     start=(eh == 0), stop=(eh == 1),
                                     skip_group_check=True)
        for dh in range(2):
            nc.vector.tensor_copy(V_sb[:, dh, :], v_ps[:, dh, :])
        V3 = V_sb.rearrange("p two (k a) -> p two k a", a=AL)

        # ---- L update ----
        for q in range(NQ):
            dl_ps = psum_sm.tile([128, R], F32, tag="t", name="t")
            for j in range(4):
                a = 4 * q + j
                for dh in range(2):
                    nc.tensor.matmul(dl_ps[32 * j:32 * j + K, :],
                                     V3[:, dh, :, a], l_rhs(a, dh),
                                     start=(dh == 0), stop=(dh == 1),
                                     tile_position=(0, 32 * j),
                                     skip_group_check=True)
            nc.vector.tensor_add(L_sb[:, q, :], L_sb[:, q, :], dl_ps)
    return O_sb


def _legalize_waits(nc, maxw=1):
    """This walrus build rejects instructions carrying more than ~1 sem wait
    ('Too many sync wait commands'). Hoist excess waits onto same-engine
    nops inserted immediately before the offending instruction."""
    blocks = list(nc.main_func.blocks)
    for bb in blocks:
        il = bb.instructions
        i = 0
        while i < len(il):
            ins = il[i]
            si = ins.sync_info
            waits = list(si.on_wait) if si and si.on_wait else []
            if len(waits) > maxw:
                keep = waits[:maxw]
                extra = waits[maxw:]
                si.on_wait = keep
                eng = ins.engine
                pos = i
                for w in extra:
                    nop = nc.engines[eng].nop(hint="wait_split")
                    tail_bb = nc.main_func.blocks[-1]
                    popped = tail_bb.instructions.pop()
                    assert popped is nop.ins
                    nsi = nop.ins.sync_info
                    if nsi is None:
                        nop.ins.sync_info = mybir.SyncInfo(on_wait=[w],
                                                           on_update=[])
                    else:
                        nsi.on_wait = [w]
                    il.insert(pos, nop.ins)
                    pos += 1
                    i += 1
            i += 1


def _build_nc():
    nc = bass.Bass("TRN2", target_bir_lowering=False, debug=False,
                   num_devices=N_CORES)
    xn_d = nc.dram_tensor("xn", [AL, 128, 256], F32, kind="ExternalInput")
    xt_d = nc.dram_tensor("xt", [AL, 2, 128, 128], F32, kind="ExternalInput")
    w1n_d = nc.dram_tensor("w1n", [HL, 2, 128, 256], F32, kind="ExternalInput")
    w1t_d = nc.dram_tensor("w1t", [HL, 2, 2, 128, 128], F32, kind="ExternalInput")
    w2n_d = nc.dram_tensor("w2n", [CL, 2, 128, 256], F32, kind="ExternalInput")
    w2t_d = nc.dram_tensor("w2t", [CL, 2, 2, 128, 128], F32, kind="ExternalInput")
    out_d = nc.dram_tensor("out", [CL, AL, 256], F32, kind="ExternalOutput")
    out1_d = nc.dram_tensor("out1", [HL, AL, 256], F32, kind="ExternalOutput")
    cc_in = nc.dram_tensor("cc_in", [HL, AL, 256], F32)
    cc_out = nc.dram_tensor("cc_out", [PH * HL, AL, 256], F32)
    groups = [[hg * PA + ag for hg in range(PH)] for ag in range(PA)]

    with (tile.TileContext(nc) if os.environ.get('PLAIN_TC') else _TC(nc)) as tc:
        from contextlib import ExitStack
        with ExitStack() as ctx:
            constp = ctx.enter_context(tc.tile_pool(name="const", bufs=1))
            psum_big = ctx.enter_context(
                tc.tile_pool(name="psb", bufs=1, space="PSUM"))
            psum_o = ctx.enter_context(
                tc.tile_pool(name="pso", bufs=4, space="PSUM"))
            psum_sm = ctx.enter_context(
                tc.tile_pool(name="pst", bufs=2, space="PSUM"))
            sb_small = ctx.enter_context(tc.tile_pool(name="sbs", bufs=4))
            pools = (psum_big, psum_o, psum_sm, sb_small)

            ident = constp.tile([128, 128], F32)
            make_identity(nc, ident)
            uni1 = constp.tile([R1, HL], F32)
            nc.vector.memset(uni1, 1.0 / R1)
            uni2 = constp.tile([R2, CL], F32)
            nc.vector.memset(uni2, 1.0 / R2)

            # ---------------- stage 1 ----------------
            with ExitStack() as s1:
                inp = s1.enter_context(tc.tile_pool(name="s1in", bufs=1))
                st = s1.enter_context(tc.tile_pool(name="s1st", bufs=1))
                xn_sb = inp.tile([128, AL, 256], F32)
                nc.sync.dma_start(out=xn_sb, in_=xn_d[:].rearrange("a r d -> r a d"))
                xt_sb = inp.tile([128, 2, AL, 128], F32)
                for dh in range(2):
                    nc.sync.dma_start(
                        out=xt_sb[:, dh, :, :],
                        in_=xt_d[:, dh].rearrange("a d r -> d a r"))
                w1n_sb = inp.tile([128, HL, 2, 256], F32)
                for dh in range(2):
                    nc.sync.dma_start(
                        out=w1n_sb[:, :, dh, :],
                        in_=w1n_d[:, dh].rearrange("k d e -> d k e"))
                w1t_sb = inp.tile([128, HL, 2, 2, 128], F32)
                for i in range(2):
                    for j in range(2):
                        nc.sync.dma_start(
                            out=w1t_sb[:, :, i, j, :],
                            in_=w1t_d[:, i, j].rearrange("k e d -> e k d"))

                state1 = {
                    "L": st.tile([128, AL // 4, R1], F32, tag="L1", name="L1"),
                    "P": st.tile([128, AL // 4, R1], F32, tag="P1", name="P1"),
                    "PT": st.tile([R1, AL // 4, 128], F32, tag="PT1", name="PT1"),
                    "S": st.tile([128, 2, AL * HL], F32, tag="S1", name="S1"),
                    "V": st.tile([128, 2, HL * AL], F32, tag="V1", name="V1"),
                    "O": st.tile([128, HL // 4, 256], F32, tag="O1", name="O1"),
                    "SQ": st.tile([AL, HL], F32, tag="SQ1", name="SQ1"),
                    "F": st.tile([AL, HL], F32, tag="F1", name="F1"),
                }
                O1 = _route_stage(
                    nc, tc, ctx, pools, HL, R1, ident, uni1,
                    lambda a, dh: xn_sb[:, a, 128 * dh:128 * dh + 128],
                    lambda a, dh: xt_sb[:, dh, a, :],
                    w1n_sb, w1t_sb, state1)
                for k in range(HL):
                    s, g = k % 4, k // 4
                    nc.sync.dma_start(out=cc_in[k],
                                      in_=O1[32 * s:32 * s + AL, g, :])
                    nc.sync.dma_start(out=out1_d[k],
                                      in_=O1[32 * s:32 * s + AL, g, :])

            nc.gpsimd.collective_compute(
                "AllGather", mybir.AluOpType.bypass,
                replica_groups=groups, ins=[cc_in[:]], outs=[cc_out[:]])

            # ---------------- stage 2 ----------------
            with ExitStack() as s2:
                inp2 = s2.enter_context(tc.tile_pool(name="s2in", bufs=1))
                st2 = s2.enter_context(tc.tile_pool(name="s2st", bufs=1))
                w2n_sb = inp2.tile([128, CL, 2, 256], F32)
                for dh in range(2):
                    nc.sync.dma_start(
                        out=w2n_sb[:, :, dh, :],
                        in_=w2n_d[:, dh].rearrange("k d e -> d k e"))
                w2t_sb = inp2.tile([128, CL, 2, 2, 128], F32)
                for i in range(2):
                    for j in range(2):
                        nc.sync.dma_start(
                            out=w2t_sb[:, :, i, j, :],
                            in_=w2t_d[:, i, j].rearrange("k e d -> e k d"))
                y_sb = inp2.tile([R2, AL, 256], F32)
                nc.sync.dma_start(out=y_sb, in_=cc_out[:])
                yt_sb = inp2.tile([128, 2, AL, R2], F32)
                for a in range(AL):
                    for dh in range(2):
                        t_ps = psum_sm.tile([128, R2], F32, tag="t", name="t")
                        nc.tensor.transpose(
                            t_ps, y_sb[:, a, 128 * dh:128 * dh + 128],
                            ident[:R2, :R2])
                        nc.vector.tensor_copy(yt_sb[:, dh, a, :], t_ps)

                state2 = {
                    "L": st2.tile([128, AL // 4, R2], F32, tag="L2", name="L2"),
                    "P": st2.tile([128, AL // 4, R2], F32, tag="P2", name="P2"),
                    "PT": st2.tile([R2, AL // 4, 128], F32, tag="PT2", name="PT2"),
                    "S": st2.tile([128, 2, AL * CL], F32, tag="S2", name="S2"),
                    "V": st2.tile([128, 2, CL * AL], F32, tag="V2", name="V2"),
                    "O": st2.tile([128, CL // 4, 256], F32, tag="O2", name="O2"),
                    "SQ": st2.tile([AL, CL], F32, tag="SQ2", name="SQ2"),
                    "F": st2.tile([AL, CL], F32, tag="F2", name="F2"),
                }
                O2 = _route_stage(
                    nc, tc, ctx, pools, CL, R2, ident, uni2,
                    lambda a, dh: y_sb[:, a, 128 * dh:128 * dh + 128],
                    lambda a, dh: yt_sb[:, dh, a, :],
                    w2n_sb, w2t_sb, state2)
                for k in range(CL):
                    s, g = k % 4, k // 4
                    nc.sync.dma_start(out=out_d[k],
                                      in_=O2[32 * s:32 * s + AL, g, :])
    _legalize_waits(nc)
    return nc


_NC_CACHE = None


def _get_nc():
    global _NC_CACHE
    if _NC_CACHE is None:
        _NC_CACHE = _build_nc()
    return _NC_CACHE


def _prep_core(x, w1, w2, hg, ag):
    xs = np.ascontiguousarray(x[ag * AL:(ag + 1) * AL])          # [AL,128,256]
    xt = np.ascontiguousarray(
        xs.transpose(0, 2, 1).reshape(AL, 2, 128, 128))
    w1s = w1[hg * HL:(hg + 1) * HL]
    w1n = np.ascontiguousarray(w1s.reshape(HL, 2, 128, 256))
    t = w1s.reshape(HL, 2, 128, 2, 128)                          # h,dh,d,eh,e
    w1t = np.ascontiguousarray(t.transpose(0, 3, 1, 4, 2))       # h,eh,dh,e,d
    w2s = w2[hg * CL:(hg + 1) * CL]
    w2n = np.ascontiguousarray(w2s.reshape(CL, 2, 128, 256))
    t2 = w2s.reshape(CL, 2, 128, 2, 128)
    w2t = np.ascontiguousarray(t2.transpose(0, 3, 1, 4, 2))
    return {"xn": xs, "xt": xt, "w1n": w1n, "w1t": w1t,
            "w2n": w2n, "w2t": w2t}


def kernel(x, route_weights1, route_weights):
    x = np.ascontiguousarray(np.asarray(x, np.float32))
    w1 = np.ascontiguousarray(np.asarray(route_weights1, np.float32))
    w2 = np.ascontiguousarray(np.asarray(route_weights, np.float32))
    nc = _get_nc()
    in_maps = []
    for core in range(N_CORES):
        hg, ag = core // PA, core % PA
        in_maps.append(_prep_core(x, w1, w2, hg, ag))
    res = run_bass_kernel_spmd(nc, in_maps, core_ids=list(range(N_CORES)))
    out = np.zeros((C, A, D), np.float32)
    for core in range(N_CORES):
        hg, ag = core // PA, core % PA
        out[hg * CL:(hg + 1) * CL, ag * AL:(ag + 1) * AL, :] = \
            res.results[core]["out"]
    return out



# revision 2
# speedup vs baseline: 44.9845x; 44.9845x over previous
"""Trainium2 Bass kernel for nn_CapsuleLayer (dynamic-routing capsule layer).

Math (reassociated so priors [K,A,R,D] are never materialized):
    stage(x_b, W, K, R):  # per (k, a) routing over R routes
      L = 0
      for it in 0..2:
        P = softmax_R(L)                          # uniform at it=0
        s[k,a,:] = sum_r P[k,a,r] * x_b[a,r,:]    # (B)  PE
        O_pre    = s @ W[k]                       # (A)  PE
        O        = O_pre * F(sq),  sq = sum_e O_pre^2   (squash)
        if it<2:
          V[k,a,:] = W[k] @ O[k,a,:]              # (V)  PE
          L[k,a,r] += sum_d x_b[a,r,d] V[k,a,d]   # (L)  PE
      return O

Sharding: 8 cores = 4 h-groups x 2 a-groups.
  Stage 1: h_loc=16 capsules, a_loc=32 batch, R=128 routes.
  AllGather of out1 among the 4 cores sharing an a-group.
  Stage 2: c_loc=8, a_loc=32, R=64 (=H) routes.
All matmuls fp32 (reduced precision breaks this net: routing softmax is
saturated, |logits| ~ 450, winner-take-all).

Layouts per core (f32):
  xn  [AL,128,256]        x natural; B-lhsT slices [r, d-half]
  xt  [AL,2,128,128]      x transposed per a, d-half: [d, r]; L-rhs
  w1n [HL,2,128,256]      W1 d-half-split: A-rhs [d,e]
  w1t [HL,2,2,128,128]    W1^T subtiles [eh][dh]: [e,d]; V-lhsT
  w2n [CL,2,128,256], w2t [CL,2,2,128,128]
Routing state q-layout: quad q=a//4, j=a%4, partition = 32*j + k (k<K).
"""

import os
import numpy as np

import concourse.bass as bass
from concourse import bacc
import concourse.mybir as mybir
import concourse.tile as tile
from concourse.masks import make_identity
from concourse.vector_clock import ScopedClock
from concourse.bass_utils import run_bass_kernel_spmd

F32 = mybir.dt.float32
AF = mybir.ActivationFunctionType
AX = mybir.AxisListType

A, B, D = 64, 128, 256
H, C = 64, 32
NIT = 3
PH, PA = 4, 2
HL, AL, CL = H // PH, A // PA, C // PH   # 16, 32, 8
N_CORES = PH * PA
R1, R2 = B, H                            # routes per stage


class _TC(tile.TileContext):
    """TileContext whose kernel-tail drain splits its sem waits across
    several SP instructions (this walrus build rejects >1 wait on a CTRL
    instruction: 'Too many sync wait commands')."""

    def _drain_and_barrier(self, tick_clock, wait_clock):
        drain_inst = self.nc.sync.drain()
        wait_clock.add_sem_waits(
            drain_inst.ins, ScopedClock({None: tick_clock.global_clock})
        )
        si = drain_inst.ins.sync_info
        waits = list(si.on_wait) if si and si.on_wait else []
        if len(waits) > 1:
            si.on_wait = waits[:1]
            for w in waits[1:]:
                nop = self.nc.sync.nop(hint="tail_wait_split")
                nsi = nop.ins.sync_info
                if nsi is None:
                    nop.ins.sync_info = mybir.SyncInfo(on_wait=[w], on_update=[])
                else:
                    nsi.on_wait = [w]
        self.nc.all_engine_barrier()
        popped = self.nc._tile_sem_poison_stack.pop()
        assert popped is self._sem_poison
        self.nc.clear_and_free_semaphores(list(self.sems.allocated().values()))
        self.nc.all_engine_barrier()


def _route_stage(nc, tc, ctx, pools, K, R, ident, uni,
                 b_lhsT, l_rhs, wn_sb, wt_sb, state):
    """Emit one routing stage. K capsules, R routes, AL batch.

    b_lhsT(a, dh) -> AP [R, 128]  (x natural slice; B stationary)
    l_rhs(a, dh)  -> AP [128, R]  (x^T slice; L moving)
    wn_sb[:, k, dh, :]    [128, 256]
    wt_sb[:, k, eh, dh, :] [128, 128]
    state: dict of persistent SBUF tiles for this stage.
    Returns O_sb tile ([128, ceil(K/4), 256]; rows 32*(k%4)+a', col-group k//4).
    """
    psum_big, psum_o, psum_sm, sb_small = pools
    NQ = AL // 4                   # quads of a
    NG = (K + 3) // 4              # k subgroups of 4
    L_sb = state["L"]              # [128, NQ, R]
    P_sb = state["P"]              # [128, NQ, R]
    PT_sb = state["PT"]            # [R, NQ, 128]
    S_sb = state["S"]              # [128, 2, AL*K]
    V_sb = state["V"]              # [128, 2, K, AL]
    O_sb = state["O"]              # [128, NG, 256]
    SQ = state["SQ"]               # [AL, K]
    F = state["F"]                 # [AL, K]

    nc.vector.memset(L_sb, 0.0)

    for it in range(NIT):
        last = it == NIT - 1
        # ---- probs ----
        if it > 0:
            for q in range(NQ):
                mx = sb_small.tile([128, 1], F32, tag="mx", name="mx")
                nmx = sb_small.tile([128, 1], F32, tag="nmx", name="nmx")
                sm = sb_small.tile([128, 1], F32, tag="sm", name="sm")
                rc = sb_small.tile([128, 1], F32, tag="rc", name="rc")
                nc.vector.reduce_max(out=mx, in_=L_sb[:, q, :], axis=AX.X)
                nc.vector.tensor_scalar_mul(nmx, mx, -1.0)
                nc.scalar.activation(out=P_sb[:, q, :], in_=L_sb[:, q, :],
                                     func=AF.Exp, bias=nmx, scale=1.0,
                                     accum_out=sm)
                nc.vector.reciprocal(rc, sm)
                nc.vector.tensor_scalar_mul(P_sb[:, q, :], P_sb[:, q, :], rc)
                pt_ps = psum_sm.tile([R, 128], F32, tag="t", name="t")
                nc.tensor.transpose(pt_ps, P_sb[:, q, :], ident)
                nc.vector.tensor_copy(PT_sb[:, q, :], pt_ps)

        # ---- B: S[dh][d, a*K+k] ----
        s_ps = psum_big.tile([128, 2, AL * K], F32, tag="big", name="big")
        for a in range(AL):
            q, j = a // 4, a % 4
            for dh in range(2):
                rhs = uni[:, :K] if it == 0 else PT_sb[:, q, 32 * j:32 * j + K]
                nc.tensor.matmul(s_ps[:, dh, a * K:(a + 1) * K],
                                 b_lhsT(a, dh), rhs,
                                 start=True, stop=True, skip_group_check=True)
        for dh in range(2):
            nc.vector.tensor_copy(S_sb[:, dh, :], s_ps[:, dh, :])
        S3 = S_sb.rearrange("p two (a k) -> p two a k", k=K)

        # ---- A + squash ----
        o_tiles = {}
        for g in range(NG):
            ks = [k for k in range(4 * g, min(4 * g + 4, K))]
            for k in ks:
                o_ps = psum_o.tile([AL, 256], F32, tag="o", name="o")
                o_tiles[k] = o_ps
                for dh in range(2):
                    nc.tensor.matmul(o_ps, S3[:, dh, :, k], wn_sb[:, k, dh, :],
                                     start=(dh == 0), stop=(dh == 1))
                scr = sb_small.tile([AL, 256], F32, tag="scr", name="scr")
                nc.scalar.activation(out=scr, in_=o_ps, func=AF.Square,
                                     accum_out=SQ[:, k:k + 1])
            g4 = slice(4 * g, 4 * g + len(ks))
            srt = sb_small.tile([AL, 4], F32, tag="srt", name="srt")
            sq1 = sb_small.tile([AL, 4], F32, tag="sq1", name="sq1")
            den = sb_small.tile([AL, 4], F32, tag="den", name="den")
            rcf = sb_small.tile([AL, 4], F32, tag="rcf", name="rcf")
            n = len(ks)
            nc.scalar.activation(out=srt[:, :n], in_=SQ[:, g4], func=AF.Sqrt)
            nc.vector.tensor_scalar_add(sq1[:, :n], SQ[:, g4], 1.0)
            nc.vector.tensor_mul(den[:, :n], sq1[:, :n], srt[:, :n])
            nc.vector.reciprocal(rcf[:, :n], den[:, :n])
            nc.vector.tensor_mul(F[:, g4], SQ[:, g4], rcf[:, :n])
            for k in ks:
                s = k % 4
                nc.scalar.activation(out=O_sb[32 * s:32 * s + AL, g, :],
                                     in_=o_tiles[k], func=AF.Copy,
                                     scale=F[:, k:k + 1])

        if last:
            break

        # ---- OT + V ----
        v_ps = psum_big.tile([128, 2, K * AL], F32, tag="big", name="big")
        for k in range(K):
            s, g = k % 4, k // 4
            ot = []
            for eh in range(2):
                t_ps = psum_sm.tile([128, AL], F32, tag="t", name="t")
                nc.tensor.transpose(
                    t_ps, O_sb[32 * s:32 * s + AL, g, 128 * eh:128 * eh + 128],
                    ident[32 * s:32 * s + AL, 32 * s:32 * s + AL],
                    tile_position=(32 * s, 0))
                ot_sb = sb_small.tile([128, AL], F32, tag="ot", name="ot")
                nc.vector.tensor_copy(ot_sb, t_ps)
                ot.append(ot_sb)
            for dh in range(2):
                for eh in range(2):
                    nc.tensor.matmul(v_ps[:, dh, k * AL:(k + 1) * AL],
                                     wt_sb[:, k, eh, dh, :], ot[eh],
                                     start=(eh == 0), stop=(eh == 1),
                                     skip_group_check=True)
        for dh in range(2):
            nc.vector.tensor_copy(V_sb[:, dh, :], v_ps[:, dh, :])
        V3 = V_sb.rearrange("p two (k a) -> p two k a", a=AL)

        # ---- L update ----
        for q in range(NQ):
            dl_ps = psum_sm.tile([128, R], F32, tag="t", name="t")
            for j in range(4):
                a = 4 * q + j
                for dh in range(2):
                    nc.tensor.matmul(dl_ps[32 * j:32 * j + K, :],
                                     V3[:, dh, :, a], l_rhs(a, dh),
                                     start=(dh == 0), stop=(dh == 1),
                                     tile_position=(0, 32 * j),
                                     skip_group_check=True)
            nc.vector.tensor_add(L_sb[:, q, :], L_sb[:, q, :], dl_ps)
    return O_sb


def _legalize_waits(nc, maxw=1):
    """This walrus build rejects instructions carrying more than ~1 sem wait
    ('Too many sync wait commands'). Hoist excess waits onto same-engine
    nops inserted immediately before the offending instruction."""
    blocks = list(nc.main_func.blocks)
    for bb in blocks:
        il = bb.instructions
        i = 0
        while i < len(il):
            ins = il[i]
            si = ins.sync_info
            waits = list(si.on_wait) if si and si.on_wait else []
            if len(waits) > maxw:
                keep = waits[:maxw]
                extra = waits[maxw:]
                si.on_wait = keep
                eng = ins.engine
                pos = i
                for w in extra:
                    nop = nc.engines[eng].nop(hint="wait_split")
                    tail_bb = nc.main_func.blocks[-1]
                    popped = tail_bb.instructions.pop()
                    assert popped is nop.ins
                    nsi = nop.ins.sync_info
                    if nsi is None:
                        nop.ins.sync_info = mybir.SyncInfo(on_wait=[w],
                                                           on_update=[])
                    else:
                        nsi.on_wait = [w]
                    il.insert(pos, nop.ins)
                    pos += 1
                    i += 1
            i += 1


def _build_nc():
    nc = bass.Bass("TRN2", target_bir_lowering=False, debug=False,
                   num_devices=N_CORES)
    xn_d = nc.dram_tensor("xn", [AL, 128, 256], F32, kind="ExternalInput")
    xt_d = nc.dram_tensor("xt", [AL, 2, 128, 128], F32, kind="ExternalInput")
    w1n_d = nc.dram_tensor("w1n", [HL, 2, 128, 256], F32, kind="ExternalInput")
    w1t_d = nc.dram_tensor("w1t", [HL, 2, 2, 128, 128], F32, kind="ExternalInput")
    w2n_d = nc.dram_tensor("w2n", [CL, 2, 128, 256], F32, kind="ExternalInput")
    w2t_d = nc.dram_tensor("w2t", [CL, 2, 2, 128, 128], F32, kind="ExternalInput")
    out_d = nc.dram_tensor("out", [CL, AL, 256], F32, kind="ExternalOutput")
    out1_d = nc.dram_tensor("out1", [HL, AL, 256], F32, kind="ExternalOutput")
    cc_in = nc.dram_tensor("cc_in", [HL, AL, 256], F32)
    cc_out = nc.dram_tensor("cc_out", [PH * HL, AL, 256], F32)
    groups = [[hg * PA + ag for hg in range(PH)] for ag in range(PA)]

    with (tile.TileContext(nc) if os.environ.get('PLAIN_TC') else _TC(nc)) as tc:
        from contextlib import ExitStack
        with ExitStack() as ctx:
            constp = ctx.enter_context(tc.tile_pool(name="const", bufs=1))
            psum_big = ctx.enter_context(
                tc.tile_pool(name="psb", bufs=1, space="PSUM"))
            psum_o = ctx.enter_context(
                tc.tile_pool(name="pso", bufs=4, space="PSUM"))
            psum_sm = ctx.enter_context(
                tc.tile_pool(name="pst", bufs=2, space="PSUM"))
            sb_small = ctx.enter_context(tc.tile_pool(name="sbs", bufs=4))
            pools = (psum_big, psum_o, psum_sm, sb_small)

            ident = constp.tile([128, 128], F32)
            make_identity(nc, ident)
            uni1 = constp.tile([R1, HL], F32)
            nc.vector.memset(uni1, 1.0 / R1)
            uni2 = constp.tile([R2, CL], F32)
            nc.vector.memset(uni2, 1.0 / R2)

            # ---------------- stage 1 ----------------
            with ExitStack() as s1:
                inp = s1.enter_context(tc.tile_pool(name="s1in", bufs=1))
                st = s1.enter_context(tc.tile_pool(name="s1st", bufs=1))
                xn_sb = inp.tile([128, AL, 256], F32)
                nc.sync.dma_start(out=xn_sb, in_=xn_d[:].rearrange("a r d -> r a d"))
                xt_sb = inp.tile([128, 2, AL, 128], F32)
                for dh in range(2):
                    nc.sync.dma_start(
                        out=xt_sb[:, dh, :, :],
                        in_=xt_d[:, dh].rearrange("a d r -> d a r"))
                w1n_sb = inp.tile([128, HL, 2, 256], F32)
                for dh in range(2):
                    nc.sync.dma_start(
                        out=w1n_sb[:, :, dh, :],
                        in_=w1n_d[:, dh].rearrange("k d e -> d k e"))
                w1t_sb = inp.tile([128, HL, 2, 2, 128], F32)
                for i in range(2):
                    for j in range(2):
                        nc.sync.dma_start(
                            out=w1t_sb[:, :, i, j, :],
                            in_=w1t_d[:, i, j].rearrange("k e d -> e k d"))

                state1 = {
                    "L": st.tile([128, AL // 4, R1], F32, tag="L1", name="L1"),
                    "P": st.tile([128, AL // 4, R1], F32, tag="P1", name="P1"),
                    "PT": st.tile([R1, AL // 4, 128], F32, tag="PT1", name="PT1"),
                    "S": st.tile([128, 2, AL * HL], F32, tag="S1", name="S1"),
                    "V": st.tile([128, 2, HL * AL], F32, tag="V1", name="V1"),
                    "O": st.tile([128, HL // 4, 256], F32, tag="O1", name="O1"),
                    "SQ": st.tile([AL, HL], F32, tag="SQ1", name="SQ1"),
                    "F": st.tile([AL, HL], F32, tag="F1", name="F1"),
                }
                O1 = _route_stage(
                    nc, tc, ctx, pools, HL, R1, ident, uni1,
                    lambda a, dh: xn_sb[:, a, 128 * dh:128 * dh + 128],
                    lambda a, dh: xt_sb[:, dh, a, :],
                    w1n_sb, w1t_sb, state1)
                for k in range(HL):
                    s, g = k % 4, k // 4
                    nc.sync.dma_start(out=cc_in[k],
                                      in_=O1[32 * s:32 * s + AL, g, :])
                    nc.sync.dma_start(out=out1_d[k],
                                      in_=O1[32 * s:32 * s + AL, g, :])

            nc.gpsimd.collective_compute(
                "AllGather", mybir.AluOpType.bypass,
                replica_groups=groups, ins=[cc_in[:]], outs=[cc_out[:]])

            # ---------------- stage 2 ----------------
            with ExitStack() as s2:
                inp2 = s2.enter_context(tc.tile_pool(name="s2in", bufs=1))
                st2 = s2.enter_context(tc.tile_pool(name="s2st", bufs=1))
                w2n_sb = inp2.tile([128, CL, 2, 256], F32)
                for dh in range(2):
                    nc.sync.dma_start(
                        out=w2n_sb[:, :, dh, :],
                        in_=w2n_d[:, dh].rearrange("k d e -> d k e"))
                w2t_sb = inp2.tile([128, CL, 2, 2, 128], F32)
                for i in range(2):
                    for j in range(2):
                        nc.sync.dma_start(
                            out=w2t_sb[:, :, i, j, :],
                            in_=w2t_d[:, i, j].rearrange("k e d -> e k d"))
                y_sb = inp2.tile([R2, AL, 256], F32)
                nc.sync.dma_start(out=y_sb, in_=cc_out[:])
                yt_sb = inp2.tile([128, 2, AL, R2], F32)
                for a in range(AL):
                    for dh in range(2):
                        t_ps = psum_sm.tile([128, R2], F32, tag="t", name="t")
                        nc.tensor.transpose(
                            t_ps, y_sb[:, a, 128 * dh:128 * dh + 128],
                            ident[:R2, :R2])
                        nc.vector.tensor_copy(yt_sb[:, dh, a, :], t_ps)

                state2 = {
                    "L": st2.tile([128, AL // 4, R2], F32, tag="L2", name="L2"),
                    "P": st2.tile([128, AL // 4, R2], F32, tag="P2", name="P2"),
                    "PT": st2.tile([R2, AL // 4, 128], F32, tag="PT2", name="PT2"),
                    "S": st2.tile([128, 2, AL * CL], F32, tag="S2", name="S2"),
                    "V": st2.tile([128, 2, CL * AL], F32, tag="V2", name="V2"),
                    "O": st2.tile([128, CL // 4, 256], F32, tag="O2", name="O2"),
                    "SQ": st2.tile([AL, CL], F32, tag="SQ2", name="SQ2"),
                    "F": st2.tile([AL, CL], F32, tag="F2", name="F2"),
                }
                O2 = _route_stage(
                    nc, tc, ctx, pools, CL, R2, ident, uni2,
                    lambda a, dh: y_sb[:, a, 128 * dh:128 * dh + 128],
                    lambda a, dh: yt_sb[:, dh, a, :],
                    w2n_sb, w2t_sb, state2)
                for k in range(CL):
                    s, g = k % 4, k // 4
                    nc.sync.dma_start(out=out_d[k],
                                      in_=O2[32 * s:32 * s + AL, g, :])
    _legalize_waits(nc)
    return nc


_NC_CACHE = None


def _get_nc():
    global _NC_CACHE
    if _NC_CACHE is None:
        _NC_CACHE = _build_nc()
    return _NC_CACHE


def _prep_core(x, w1, w2, hg, ag):
    xs = np.ascontiguousarray(x[ag * AL:(ag + 1) * AL])          # [AL,128,256]
    xt = np.ascontiguousarray(
        xs.transpose(0, 2, 1).reshape(AL, 2, 128, 128))
    w1s = w1[hg * HL:(hg + 1) * HL]
    w1n = np.ascontiguousarray(w1s.reshape(HL, 2, 128, 256))
    t = w1s.reshape(HL, 2, 128, 2, 128)                          # h,dh,d,eh,e
    w1t = np.ascontiguousarray(t.transpose(0, 3, 1, 4, 2))       # h,eh,dh,e,d
    w2s = w2[hg * CL:(hg + 1) * CL]
    w2n = np.ascontiguousarray(w2s.reshape(CL, 2, 128, 256))
    t2 = w2s.reshape(CL, 2, 128, 2, 128)
    w2t = np.ascontiguousarray(t2.transpose(0, 3, 1, 4, 2))
    return {"xn": xs, "xt": xt, "w1n": w1n, "w1t": w1t,
            "w2n": w2n, "w2t": w2t}


def _make_in_map(x, w1, w2, core):
    hg, ag = core // PA, core % PA
    return _prep_core(x, w1, w2, hg, ag)


def _assemble(results):
    out = np.zeros((C, A, D), np.float32)
    for core in range(N_CORES):
        hg, ag = core // PA, core % PA
        out[hg * CL:(hg + 1) * CL, ag * AL:(ag + 1) * AL, :] = \
            results[core]["out"]
    return out


def kernel(x, route_weights1, route_weights):
    x = np.ascontiguousarray(np.asarray(x, np.float32))
    w1 = np.ascontiguousarray(np.asarray(route_weights1, np.float32))
    w2 = np.ascontiguousarray(np.asarray(route_weights, np.float32))
    nc = _get_nc()
    in_maps = [_make_in_map(x, w1, w2, core) for core in range(N_CORES)]
    res = run_bass_kernel_spmd(nc, in_maps, core_ids=list(range(N_CORES)))
    return _assemble(res.results)

